# revision 18
# baseline (speedup 1.0000x reference)
"""Trainium2 Bass/Tile kernel: two chained VALID 3x3 convolutions.

    x  [N,3,256,256] --conv(w1)--> h [N,64,254,254] --conv(w2)--> out [N,128,252,252]

Data-parallel over 8 NeuronCores: batch N=16 -> 2 images per core, conv
weights replicated.  Per core the convs are computed as implicit GEMMs on the
tensor engine.  The kernel is tensor-engine issue-rate bound (the HW activity
monitor duty-cycles the PE between 2.4 GHz and 1.2 GHz column rates under
sustained load), so the design minimizes total matmul *columns*.

conv1 ("slot" layout, 0.5 columns per h pixel):
  One K=54 matmul per EVEN h-row j computes [h_j ; h_{j+1}] stacked across
  the 128 PSUM partitions (block-diagonal stationary matrix).  A single
  full-width DVE cast writes PSUM -> H[:, j, :], which is exactly the
  [A; B] layout conv2 consumes: H[0:64, j] = h_j, H[64:128, j] = h_{j+1}.
  ODD slots are filled with two SBUF half-copies (bf16 2x): DVE copies
  h_{j-1} from the B-half of slot j-2, scalar copies h_j from the A-half
  of slot j.

conv2: contraction over C1*9=576 in 5 matmul passes per 2-row chunk:
    H  pairs (0,dj)+(1,dj), dj=0..2, K=128                   (3 passes)
    H2 = [D; A2]: D = h shifted left 1 col, A2 = h copy
         pair (2,1)+(2,0) at K=128                           (1 pass)
         single (2,2) via D shifted one more col, K=64       (1 pass)
  A2 is built with contiguous bulk SBUF->SBUF DMAs; D on the compute
  engines (bf16 2x).  PSUM accumulates the 5 matmuls, scalar engine casts
  to SBUF (bf16), DMA to HBM; host casts the bf16 output back to fp32.

H and H2 are each SPLIT at the even slot MID into half-strip tiles
(chunks never straddle the split), so the early conv2 chunks of a strip
depend only on the first half -- the tail writes of the second half can
finish while the next strip's conv2 is already running, eliminating the
strip-transition stalls of a whole-tile dependency.

Emission order == tensor-engine execution order: conv1 of strip s+1 and
the im2col DMAs of strip s+2 are interleaved between the conv2 chunks of
strip s (conv1 front-loaded) so every producer runs ahead of its consumer
even at the boosted (k=8) tensor cadence.
"""

from contextlib import ExitStack

import ml_dtypes
import numpy as np

import concourse.bass as bass
import concourse.mybir as mybir
import concourse.tile as tile
import concourse.bass_utils as bass_utils
from concourse import bacc

N_CORES = 8
FULL_N = 16
C0, C1, C2 = 3, 64, 128

MODE = "bf16"


def _mm_dt():
    return mybir.dt.bfloat16 if MODE == "bf16" else mybir.dt.float32r


def _np_dt():
    return ml_dtypes.bfloat16 if MODE == "bf16" else np.float32


class Geom:
    def __init__(self, npc, h0, w0, ty):
        self.npc = npc          # images per core
        self.h0, self.w0 = h0, w0
        self.h1, self.w1 = h0 - 2, w0 - 2
        self.h2, self.w2 = h0 - 4, w0 - 4
        self.ty = ty            # conv2 output rows per strip
        assert ty % 2 == 0 and self.h2 % ty == 0
        self.mid = (ty // 2 + 1) // 2 * 2  # even split slot


GEOM = Geom(npc=FULL_N // N_CORES, h0=256, w0=256, ty=42)


def _piece_edges(ty, mid):
    """Even slot values splitting [0, ty] into ~5 windows for the bulk
    copies, always including the tile-split slot `mid` and a SMALL window
    right after it -- the first H2b rows gate conv2 chunk MID, so they
    must arrive in a short transfer."""
    e1 = max(2, mid // 2 // 2 * 2)
    e3 = min(ty, mid + 4)
    e4 = min(ty - 2, (e3 + ty + 1) // 2 // 2 * 2)
    edges = sorted({0, e1, mid, e3, e4, ty})
    assert all(e % 2 == 0 for e in edges)
    return edges


def _emit(ctx: ExitStack, tc: tile.TileContext, g: Geom, out, x, w1d, w2p, w2q,
          w2r, mm_dt):
    nc = tc.nc
    f32 = mybir.dt.float32
    Copy = mybir.ActivationFunctionType.Copy
    TY, W1, W2, MID = g.ty, g.w1, g.w2, g.mid

    wpool = ctx.enter_context(tc.tile_pool(name="weights", bufs=1))
    b1pool = ctx.enter_context(tc.tile_pool(name="b1", bufs=3))
    hapool = ctx.enter_context(tc.tile_pool(name="ha", bufs=2))
    hbpool = ctx.enter_context(tc.tile_pool(name="hb", bufs=2))
    h2apool = ctx.enter_context(tc.tile_pool(name="h2a", bufs=2))
    h2bpool = ctx.enter_context(tc.tile_pool(name="h2b", bufs=2))
    opool = ctx.enter_context(tc.tile_pool(name="o2", bufs=6))
    ps1 = ctx.enter_context(tc.tile_pool(name="ps1", bufs=3, space="PSUM"))
    ps2 = ctx.enter_context(tc.tile_pool(name="ps2", bufs=4, space="PSUM"))

    w1d_sb = wpool.tile([54, 128], mm_dt)
    nc.sync.dma_start(w1d_sb[:], w1d)
    w2p_sb = wpool.tile([128, 3, C2], mm_dt)
    nc.sync.dma_start(w2p_sb[:], w2p)
    w2q_sb = wpool.tile([128, C2], mm_dt)
    nc.sync.dma_start(w2q_sb[:], w2q)
    w2r_sb = wpool.tile([C1, C2], mm_dt)
    nc.sync.dma_start(w2r_sb[:], w2r)

    def im2col(n, y0):
        """Allocate B1 for a strip; return thunks that emit its 18 DMAs.

        Partition (di*3+dj)*3+c    holds x[c, y0+rr+di,   dj : dj+W1]
        Partition 27+(di*3+dj)*3+c holds x[c, y0+rr+1+di, dj : dj+W1]
        for slot index rr = 0..TY (moving column rr -> h rows rr, rr+1).
        """
        B1 = b1pool.tile([54, TY + 1, W1], mm_dt, tag="b1")

        def dma(t9):
            di, dj = divmod(t9, 3)
            nc.sync.dma_start(
                B1[3 * t9:3 * t9 + 3],
                x[n, :, y0 + di:y0 + di + TY + 1, dj:dj + W1])
            nc.sync.dma_start(
                B1[27 + 3 * t9:27 + 3 * t9 + 3],
                x[n, :, y0 + 1 + di:y0 + 1 + di + TY + 1, dj:dj + W1])
        return B1, [lambda t9=t9: dma(t9) for t9 in range(9)]

    class Hpack:
        """H (slot layout) and H2, each split at MID into half-strip
        tiles.  slot(j)/row(m) return (tile, local index)."""

        def __init__(self):
            self.Ha = hapool.tile([128, MID, W1], mm_dt, tag="ha")
            self.Hb = hbpool.tile([128, TY + 1 - MID, W1], mm_dt, tag="hb")
            self.H2a = h2apool.tile([128, MID, W1], mm_dt, tag="h2a")
            self.H2b = h2bpool.tile([128, TY + 2 - MID, W1], mm_dt, tag="h2b")

        def slot(self, j):
            return (self.Ha, j) if j < MID else (self.Hb, j - MID)

        def row2(self, m):
            return (self.H2a, m) if m < MID else (self.H2b, m - MID)

    def conv1_chunk(B1, hp, j):
        """Even slot j: one K=54/M=128 matmul producing [h_j ; h_{j+1}],
        one full-width DVE cast, then fill ODD slot j-1 = [h_{j-1}; h_j]
        with two SBUF half-copies."""
        P1 = ps1.tile([128, W1], f32, tag="p1")
        nc.tensor.matmul(P1[:], w1d_sb[:], B1[:, j, :],
                         start=True, stop=True)
        Tj, ij = hp.slot(j)
        nc.vector.tensor_copy(Tj[:, ij, :], P1[:])
        if j >= 2:
            Td, idd = hp.slot(j - 1)
            Ts, iss = hp.slot(j - 2)
            nc.vector.tensor_copy(Td[0:C1, idd, :], Ts[C1:128, iss, :])
            nc.scalar.activation(Td[C1:128, idd, :], Tj[0:C1, ij, :], Copy)

    def _ranges_split(a, b):
        """Split [a, b] at the MID boundary."""
        out = []
        if a < MID:
            out.append((a, min(b, MID - 1)))
        if b >= MID:
            out.append((max(a, MID), b))
        return [(lo, hi) for lo, hi in out if hi >= lo]

    def conv1_piece(hp, jlo, jhi, pidx):
        """A2/D copies for h rows (jlo, jhi], clipped to [2, TY] (emitted
        after slot jhi's cast and odd-fill).  A2: contiguous bulk
        SBUF->SBUF DMA; D: compute engines (bf16 2x), DVE for the first
        half's windows, scalar for the second half's."""
        for a, b in _ranges_split(max(jlo + 1, 2), jhi):
            Ts, _ = hp.slot(a)      # source rows a..b share one H half
            T2, _ = hp.row2(a)
            ia, ib = (a, b) if a < MID else (a - MID, b - MID)
            nc.sync.dma_start(T2[C1:128, ia:ib + 1, :], Ts[0:C1, ia:ib + 1, :])
            src = Ts[0:C1, ia:ib + 1, 1:W1]
            dst = T2[0:C1, ia:ib + 1, 0:W1 - 1]
            if pidx < 2:
                nc.vector.tensor_copy(dst, src)
            else:
                nc.scalar.activation(dst, src, Copy)
        if jhi == TY:
            # final row TY+1 = B-half of slot TY
            T2, i2 = hp.row2(TY + 1)
            Ts, iss = hp.slot(TY)
            nc.sync.dma_start(T2[C1:128, i2, :], Ts[C1:128, iss, :])
            nc.scalar.activation(T2[0:C1, i2, 0:W1 - 1],
                                 Ts[C1:128, iss, 1:W1], Copy)

    def conv2_chunk(n, y0, hp, t):
        Ht, it = hp.slot(t)         # slots t, t+1 never straddle MID
        H2t, im = hp.row2(t + 2)    # rows t+2, t+3 never straddle MID
        P2 = ps2.tile([C2, 2, W2], f32, tag="p2")
        for dj in range(3):  # pairs: taps (0,dj) + (1,dj), K=128
            nc.tensor.matmul(P2[:], w2p_sb[:, dj, :],
                             Ht[:, it:it + 2, dj:dj + W2],
                             start=(dj == 0), stop=False)
        # pair: taps (2,1) [D] + (2,0) [A2], K=128
        nc.tensor.matmul(P2[:], w2q_sb[:],
                         H2t[:, im:im + 2, 0:W2],
                         start=False, stop=False)
        # single: tap (2,2) via D shifted one more col, K=64
        nc.tensor.matmul(P2[:], w2r_sb[:],
                         H2t[0:C1, im:im + 2, 1:1 + W2],
                         start=False, stop=True)
        O2 = opool.tile([C2, 2, W2], mm_dt, tag="o2")
        # PSUM->SBUF out-cast on the scalar (Act) engine; DVE owns the
        # conv1 casts
        nc.scalar.activation(O2[:], P2[:], Copy)
        nc.sync.dma_start(out[n, :, y0 + t:y0 + t + 2, :], O2[:])

    EDGES = _piece_edges(TY, MID)

    def conv1_work(B1, hp):
        work = []
        ei = 1
        for j in range(0, TY + 2, 2):
            work.append(lambda j=j: conv1_chunk(B1, hp, j))
            if ei < len(EDGES) and j == EDGES[ei]:
                jlo, jhi = EDGES[ei - 1], EDGES[ei]
                work.append(lambda jlo=jlo, jhi=jhi, pidx=ei - 1:
                            conv1_piece(hp, jlo, jhi, pidx))
                ei += 1
        return work

    strips = [(n, y0) for n in range(g.npc) for y0 in range(0, g.h2, TY)]
    ns = len(strips)

    # prologue: load strip 0, run conv1(0) as a burst, start loading strip 1
    B1_0, dmas = im2col(*strips[0])
    for t in dmas:
        t()
    cur = Hpack()
    for w in conv1_work(B1_0, cur):
        w()
    B1s = {}
    if ns > 1:
        B1_1, dmas = im2col(*strips[1])
        for t in dmas:
            t()
        B1s[1] = B1_1

    # steady state: conv2(i) interleaved with conv1(i+1) and im2col(i+2),
    # conv1 front-loaded so its last copies land before the next strip's
    # conv2 needs them
    for i in range(ns):
        n, y0 = strips[i]
        c1work = []
        nxt = None
        if i + 1 < ns:
            nxt = Hpack()
            c1work = conv1_work(B1s.pop(i + 1), nxt)
        imwork = []
        if i + 2 < ns:
            B1x, imwork = im2col(*strips[i + 2])
            B1s[i + 2] = B1x
        c2work = [lambda t=t: conv2_chunk(n, y0, cur, t)
                  for t in range(0, TY, 2)]

        EXTRA = max(len(c1work) - len(c2work) + 2, 0)
        ci = 0
        for t in range(len(c2work)):
            per = 2 if t < EXTRA else 1
            for _ in range(per):
                if ci < len(c1work):
                    c1work[ci]()
                    ci += 1
            c2work[t]()
            if t < len(imwork):
                imwork[t]()
        while ci < len(c1work):
            c1work[ci]()
            ci += 1
        cur = nxt


def build(g: Geom = GEOM, mm_dt=None):
    if mm_dt is None:
        mm_dt = _mm_dt()
    nc = bacc.Bacc("TRN2", target_bir_lowering=False, debug=False,
                   num_devices=N_CORES)
    x = nc.dram_tensor("x", [g.npc, C0, g.h0, g.w0], mm_dt,
                       kind="ExternalInput").ap()
    w1d = nc.dram_tensor("w1d", [54, 128], mm_dt, kind="ExternalInput").ap()
    w2p = nc.dram_tensor("w2p", [128, 3, C2], mm_dt, kind="ExternalInput").ap()
    w2q = nc.dram_tensor("w2q", [128, C2], mm_dt, kind="ExternalInput").ap()
    w2r = nc.dram_tensor("w2r", [C1, C2], mm_dt, kind="ExternalInput").ap()
    out = nc.dram_tensor("out", [g.npc, C2, g.h2, g.w2], mm_dt,
                         kind="ExternalOutput").ap()
    with tile.TileContext(nc) as tc:
        with ExitStack() as ctx:
            _emit(ctx, tc, g, out, x, w1d, w2p, w2q, w2r, mm_dt)
    nc.compile()
    return nc


def host_round(a: np.ndarray) -> np.ndarray:
    """Cast fp32 to the matmul storage dtype (bf16 cast, or tf32 rounding)."""
    a = np.ascontiguousarray(a, dtype=np.float32)
    if MODE == "bf16":
        return a.astype(ml_dtypes.bfloat16)
    b = a.view(np.uint32).copy()
    b += 0xFFF + ((b >> 13) & 1)
    b &= np.uint32(0xFFFFE000)
    return b.view(np.float32)


def pack_weights(w1: np.ndarray, w2: np.ndarray):
    """Host-side repack so every device DMA is contiguous.

    w1d: block-diagonal [54, 128]: w1d[p, o] = w1t[p, o]; w1d[27+p, 64+o] =
         w1t[p, o], where w1t[p, o] = w1[o, c, di, dj], p = (di*3+dj)*3 + c
    w2p[k, dj, o]: k<64 -> w2[o, k, 0, dj];  k>=64 -> w2[o, k-64, 1, dj]
    w2q[k, o]:     k<64 -> w2[o, k, 2, 1];   k>=64 -> w2[o, k-64, 2, 0]
    w2r[c, o] = w2[o, c, 2, 2]
    """
    w1 = np.ascontiguousarray(np.asarray(w1), dtype=np.float32)
    w2 = np.ascontiguousarray(np.asarray(w2), dtype=np.float32)
    w1t = w1.transpose(2, 3, 1, 0).reshape(27, C1)
    w1d = np.zeros((54, 128), np.float32)
    w1d[0:27, 0:C1] = w1t
    w1d[27:54, C1:128] = w1t
    w2p = np.empty((128, 3, C2), np.float32)
    w2p[:C1] = w2[:, :, 0, :].transpose(1, 2, 0)
    w2p[C1:] = w2[:, :, 1, :].transpose(1, 2, 0)
    w2q = np.empty((128, C2), np.float32)
    w2q[:C1] = w2[:, :, 2, 1].transpose(1, 0)
    w2q[C1:] = w2[:, :, 2, 0].transpose(1, 0)
    w2r = np.ascontiguousarray(w2[:, :, 2, 2].transpose(1, 0))
    return (host_round(w1d), host_round(w2p), host_round(w2q),
            host_round(w2r))


_NC_CACHE: dict = {}


def _get_nc():
    key = ("main", MODE)
    if key not in _NC_CACHE:
        _NC_CACHE[key] = build()
    return _NC_CACHE[key]


def run(x, w1, w2, trace: bool = False):
    """Shard, run on 8 cores, gather.  Returns (out, BassKernelResults)."""
    x = np.ascontiguousarray(np.asarray(x), dtype=np.float32)
    assert x.shape == (FULL_N, C0, GEOM.h0, GEOM.w0), x.shape
    w1d, w2p, w2q, w2r = pack_weights(w1, w2)
    xs = host_round(x).reshape(N_CORES, GEOM.npc, C0, GEOM.h0, GEOM.w0)
    in_maps = [
        {"x": np.ascontiguousarray(xs[c]), "w1d": w1d, "w2p": w2p,
         "w2q": w2q, "w2r": w2r}
        for c in range(N_CORES)
    ]
    nc = _get_nc()
    res = bass_utils.run_bass_kernel_spmd(
        nc, in_maps, core_ids=list(range(N_CORES)), trace=trace)
    out = np.concatenate([r["out"] for r in res.results], axis=0)
    return out.astype(np.float32), res


def kernel(x, w1, w2):
    out, _ = run(x, w1, w2, trace=False)
    return out
